# revision 3
# baseline (speedup 1.0000x reference)
"""GCNEncoder (3x GraphConv, D=64) on 8 Trainium2 NeuronCores.

Strategy (v2 — transfer-minimized; the axon tunnel at ~70MB/s h2d and
~30MB/s d2h dominates wall-clock, so every input is shipped once, small,
and in the narrowest dtype that keeps rel-err under tolerance):
  - Host: dedup edges, relabel nodes by in-degree (descending), partition the
    relabeled dst nodes into 128-row blocks dealt round-robin across 8 cores,
    and build a block-ELL structure (per dst-block: K_j neighbor slots per
    node, uniform across cores so a single SPMD program works).
  - Per-core inputs: the core's own x shard (node-major, bf16), gather-token
    indices in non-replicated [16, T/16] form (replicated to 128 partitions
    on device), ELL edge weights (bf16), the six 64x64 weight matrices and
    three pre-broadcast [128,64] biases (f32). Output is bf16.
  - Linearity: agg @ W_rel == segment_sum(w * (h @ W_rel)[src]), so each layer
    keeps a node-major table y = h @ W_rel in HBM, and the aggregation output
    plus the root term r = h @ W_root + b is already the layer output.
  - Device prologue (layer-1 dense part): per block, transpose the x block on
    the PE, then y1 = x@W_rel1 and r1 = x@W_root1 + b1 via two matmuls; an
    AllGather builds the full f32 y table (the table never crosses the
    tunnel, so it stays f32 — also required: dma_gather elem must be a
    multiple of 256B).
  - Device, per layer: per dst-block, an indirect DMA gathers the K_j neighbor
    rows per partition from the y table (one dma_gather per 32768-row window;
    gpsimd gather indices are int16); DVE multiplies by edge weights
    (broadcast along features) and reduces over K; add the resident r term;
    for layers 1-2, ReLU + two 64x64 matmuls (node-major direct) produce the
    next layer's y/r and an AllGather rebuilds the y table; layer 3 writes
    bf16 output.
"""

import os

import numpy as np

P = 128
D = 64
NCORES = 8


# ---------------------------------------------------------------- host prep


def _preprocess(x, edge_index, edge_weight):
    import ml_dtypes

    bf16 = ml_dtypes.bfloat16
    N = x.shape[0]
    src = np.asarray(edge_index[0], dtype=np.int64)
    dst = np.asarray(edge_index[1], dtype=np.int64)
    w = np.asarray(edge_weight, dtype=np.float64)

    # dedup parallel edges (sum weights)
    key = dst * N + src
    ukey, inv = np.unique(key, return_inverse=True)
    uw = np.bincount(inv, weights=w).astype(np.float32)
    udst = (ukey // N).astype(np.int64)
    usrc = (ukey % N).astype(np.int64)

    deg = np.bincount(udst, minlength=N)

    # per-core block count
    B = -(-N // (NCORES * P))  # ceil
    Npad = NCORES * B * P

    # order nodes by degree desc; sorted position t -> orig node order[t]
    order = np.argsort(-deg, kind="stable")
    order_pad = np.concatenate([order, np.full(Npad - N, -1, dtype=np.int64)])

    # sorted block g = j*NCORES + c  ->  core c, slot j
    # new id layout: new = c*B*P + j*P + p  where sorted pos t = g*P + p
    t = np.arange(Npad)
    g = t // P
    p = t % P
    c = g % NCORES
    j = g // NCORES
    newpos_of_sorted = c * (B * P) + j * P + p
    # perm: new id -> orig node (-1 for dummy)
    perm = np.empty(Npad, dtype=np.int64)
    perm[newpos_of_sorted] = order_pad
    # inv_new: orig node -> new id
    sorted_pos = np.empty(N, dtype=np.int64)
    sorted_pos[order] = np.arange(N)
    inv_new = newpos_of_sorted[sorted_pos]

    # dma_gather indices are signed int16, so the table is addressed through
    # four 32768-row windows; per (block slot j, window w) the neighbor count
    # is padded to the max over all cores/dsts of that slot (uniform SPMD).
    WIN = 32768
    NW = -(-Npad // WIN)
    nd = inv_new[udst]  # new dst id per edge
    ns = inv_new[usrc]  # new src id per edge
    wid = ns // WIN

    ej_all = (nd % (B * P)) // P
    ep_all = nd % P
    ec_all = nd // (B * P)
    # counts per (core, slot j, partition, window)
    cnt = np.zeros((NCORES, B, P, NW), dtype=np.int64)
    np.add.at(cnt, (ec_all, ej_all, ep_all, wid), 1)
    K_jw = cnt.max(axis=(0, 2))  # [B, NW]
    if K_jw.sum() == 0:
        K_jw[:, 0] = 1
    # ensure at least one column per block (so g tile is non-empty)
    K_jw[:, 0] = np.maximum(K_jw[:, 0], 1)
    K_j = K_jw.sum(axis=1)  # [B] total columns per block
    off_j = np.concatenate([[0], np.cumsum(K_j)])
    off_jw = np.concatenate(
        [np.zeros((B, 1), np.int64), np.cumsum(K_jw, axis=1)], axis=1
    ) + off_j[:-1, None]
    K_total = int(off_j[-1])

    # rank of each edge within its (dst, window) group
    gkey = nd * NW + wid
    eorder = np.argsort(gkey, kind="stable")
    gk_s = gkey[eorder]
    nd_s = nd[eorder]
    wid_s = wid[eorder]
    ns_s = ns[eorder]
    w_s = uw[eorder]
    first = np.concatenate([[True], gk_s[1:] != gk_s[:-1]])
    gid = np.cumsum(first) - 1
    gstart = np.nonzero(first)[0]
    k_within = np.arange(len(gk_s)) - gstart[gid]

    ec = nd_s // (B * P)
    rem = nd_s % (B * P)
    ej = rem // P
    ep = rem % P
    col = off_jw[ej, wid_s] + k_within

    ell_idx = np.zeros((NCORES, P, K_total), dtype=np.int16)  # window-local
    ell_w = np.zeros((NCORES, P, K_total), dtype=np.float32)
    ell_idx[ec, ep, col] = (ns_s % WIN).astype(np.int16)
    ell_w[ec, ep, col] = w_s

    # token-format (wrapped int16) index arrays for dma_gather:
    # per (j, w): tokens t = k*128 + p over its column range; wrapped
    # [16, ntok/16]. Shipped non-replicated; the device copies the 16-row
    # strip to all eight 16-partition gpsimd groups.
    ntok_jw = K_jw * P
    tok_cum = np.concatenate([[0], np.cumsum(ntok_jw.reshape(-1))])
    TOK_TOTAL = int(tok_cum[-1])
    idx_tok = np.zeros((NCORES, 16, TOK_TOTAL // 16), dtype=np.int16)
    for j in range(B):
        for w in range(NW):
            K = int(K_jw[j, w])
            if K == 0:
                continue
            c0 = int(off_jw[j, w])  # absolute col start
            t0 = int(tok_cum[j * NW + w])
            ntok = K * P
            # tokens [K, P] -> linear (k*128+p) -> wrap [ntok/16, 16] -> T
            blk = ell_idx[:, :, c0 : c0 + K]  # [NCORES, P, K]
            lin = blk.transpose(0, 2, 1).reshape(NCORES, ntok)  # t = k*128+p
            wrapped = lin.reshape(NCORES, ntok // 16, 16).transpose(0, 2, 1)
            idx_tok[:, :, t0 // 16 : (t0 + ntok) // 16] = wrapped

    # per-core x shard, node-major, bf16: x_arr[c, j*P+p, f] = x[perm[...]]
    real = perm >= 0
    x_new = np.zeros((Npad, D), dtype=np.float32)
    x_new[real] = np.asarray(x, dtype=np.float32)[perm[real]]
    x_arr = np.ascontiguousarray(
        x_new.reshape(NCORES, B * P, D).astype(bf16)
    )

    return dict(
        N=N,
        B=B,
        Npad=Npad,
        WIN=WIN,
        NW=NW,
        perm=perm,
        K_j=K_j,
        off_j=off_j,
        K_jw=K_jw,
        off_jw=off_jw,
        tok_cum=tok_cum,
        TOK_TOTAL=TOK_TOTAL,
        K_total=K_total,
        idx_tok=idx_tok,
        ell_w=ell_w.astype(bf16),
        x_arr=x_arr,
    )


# ---------------------------------------------------------------- bass build


def _build(prep):
    import concourse.bacc as bacc
    import concourse.mybir as mybir
    import concourse.tile as tile
    from concourse.masks import make_identity

    f32 = mybir.dt.float32
    bf = mybir.dt.bfloat16
    i16 = mybir.dt.int16
    B = prep["B"]
    Npad = prep["Npad"]
    WIN = prep["WIN"]
    NW = prep["NW"]
    K_j = prep["K_j"]
    off_j = prep["off_j"]
    K_jw = prep["K_jw"]
    off_jw = prep["off_jw"]
    tok_cum = prep["tok_cum"]
    TOK_TOTAL = prep["TOK_TOTAL"]
    K_total = prep["K_total"]

    nc = bacc.Bacc(
        "TRN2",
        target_bir_lowering=False,
        debug=False,
        num_devices=NCORES,
    )

    # IO
    x_in = nc.dram_tensor("xcore", [B * P, D], bf, kind="ExternalInput")
    idx_in = nc.dram_tensor("idx_tok", [16, TOK_TOTAL // 16], i16, kind="ExternalInput")
    w_in = nc.dram_tensor("ell_w", [P, K_total], bf, kind="ExternalInput")
    wmat_in = {
        nm: nc.dram_tensor(nm, [D, D], f32, kind="ExternalInput")
        for nm in ("W_rel1", "W_root1", "W_rel2", "W_root2", "W_rel3", "W_root3")
    }
    bb_in = {
        nm: nc.dram_tensor(nm, [P, D], f32, kind="ExternalInput")
        for nm in ("b1", "b2", "b3")
    }
    out_t = nc.dram_tensor("h3", [B * P, D], bf, kind="ExternalOutput")

    with tile.TileContext(nc) as tc:
        with (
            tc.tile_pool(name="const", bufs=1) as cpool,
            tc.tile_pool(name="dram", bufs=1, space="DRAM") as dpool,
            tc.tile_pool(name="gather", bufs=3) as gpool,
            tc.tile_pool(name="work", bufs=4) as wpool,
            tc.tile_pool(name="psum", bufs=1, space="PSUM") as ppool,
        ):
            # residents
            idx_res = cpool.tile([P, TOK_TOTAL // 16], i16, tag="idx")
            w_bf = cpool.tile([P, K_total], bf, tag="wbf")
            w_res = cpool.tile([P, K_total], f32, tag="w")
            r_res = cpool.tile([P, B * D], f32, tag="r")
            ident = cpool.tile([P, P], f32, tag="ident")
            Wt = {k: cpool.tile([D, D], f32, tag=k, name=k) for k in wmat_in}
            bt = {k: cpool.tile([P, D], f32, tag=k, name=k) for k in bb_in}

            # replicate the 16-row token strip to all 8 gpsimd groups
            for grp in range(8):
                nc.sync.dma_start(
                    out=idx_res[16 * grp : 16 * (grp + 1), :], in_=idx_in.ap()
                )
            nc.sync.dma_start(out=w_bf[:], in_=w_in.ap())
            nc.scalar.activation(
                out=w_res[:], in_=w_bf[:], func=mybir.ActivationFunctionType.Copy
            )
            for k in Wt:
                nc.sync.dma_start(out=Wt[k][:], in_=wmat_in[k].ap())
            for k in bt:
                nc.sync.dma_start(out=bt[k][:], in_=bb_in[k].ap())
            make_identity(nc, ident[:])

            # DRAM: y table + own-shard staging
            table2 = dpool.tile([Npad, D], f32, tag="table")
            y_own = dpool.tile([B * P, D], f32, tag="yown")

            # ---------------- prologue: y1 = x@W_rel1, r1 = x@W_root1 + b1
            for jb in range(B):
                xb = wpool.tile([P, D], bf, tag="xb")
                nc.sync.dma_start(
                    out=xb[:], in_=x_in.ap()[jb * P : (jb + 1) * P, :]
                )
                xf = wpool.tile([P, D], f32, tag="xf")
                nc.scalar.activation(
                    out=xf[:], in_=xb[:], func=mybir.ActivationFunctionType.Copy
                )
                xTp = ppool.tile([D, P], f32, tag="preT", bufs=2)
                nc.tensor.transpose(out=xTp[:], in_=xf[:], identity=ident[:])
                xT = wpool.tile([D, P], f32, tag="hT")
                nc.scalar.activation(
                    out=xT[:], in_=xTp[:], func=mybir.ActivationFunctionType.Copy
                )
                yp = ppool.tile([P, D], f32, tag="ynp", bufs=2)
                nc.tensor.matmul(
                    out=yp[:], lhsT=xT[:], rhs=Wt["W_rel1"][:], start=True, stop=True
                )
                ys = wpool.tile([P, D], f32, tag="ysb")
                nc.scalar.activation(
                    out=ys[:], in_=yp[:], func=mybir.ActivationFunctionType.Copy
                )
                nc.sync.dma_start(out=y_own[jb * P : (jb + 1) * P, :], in_=ys[:])
                rp = ppool.tile([P, D], f32, tag="rnp", bufs=2)
                nc.tensor.matmul(
                    out=rp[:], lhsT=xT[:], rhs=Wt["W_root1"][:], start=True, stop=True
                )
                nc.vector.tensor_add(
                    out=r_res[:, jb * D : (jb + 1) * D], in0=rp[:], in1=bt["b1"][:]
                )

            nc.gpsimd.collective_compute(
                "AllGather",
                mybir.AluOpType.bypass,
                replica_groups=[list(range(NCORES))],
                ins=[y_own[:].opt()],
                outs=[table2[:].opt()],
            )

            # ---------------- 3 gather/aggregate layers
            for layer in (1, 2, 3):
                W_rel_nxt = Wt[f"W_rel{layer + 1}"] if layer < 3 else None
                W_root_nxt = Wt[f"W_root{layer + 1}"] if layer < 3 else None
                b_nxt = bt[f"b{layer + 1}"] if layer < 3 else None

                for jb in range(B):
                    K = int(K_j[jb])
                    off = int(off_j[jb])
                    g = gpool.tile([P, K * D], f32, tag="g")
                    # one dma_gather per 32768-row table window
                    for wnd in range(NW):
                        Kw = int(K_jw[jb, wnd])
                        if Kw == 0:
                            continue
                        cw = int(off_jw[jb, wnd]) - off
                        ntok = Kw * P
                        t0 = int(tok_cum[jb * NW + wnd])
                        r0 = wnd * WIN
                        r1 = min(Npad, (wnd + 1) * WIN)
                        nc.gpsimd.dma_gather(
                            out_ap=g[:, cw * D : (cw + Kw) * D].rearrange(
                                "p (c e) -> p c e", e=D
                            ),
                            in_ap=table2[r0:r1, :],
                            idxs_ap=idx_res[:, t0 // 16 : (t0 + ntok) // 16],
                            num_idxs=ntok,
                            num_idxs_reg=ntok,
                            elem_size=D,
                            single_packet=False,
                        )
                    # g *= w (broadcast along feature dim)
                    g3 = g[:].rearrange("p (k f) -> p k f", f=D)
                    wb = w_res[:, off : off + K].unsqueeze(-1).to_broadcast([P, K, D])
                    nc.vector.tensor_tensor(
                        out=g3, in0=g3, in1=wb, op=mybir.AluOpType.mult
                    )
                    # agg[p, f] = sum_k g[p, k, f]
                    agg = wpool.tile([P, D], f32, tag="agg")
                    gT = g[:].rearrange("p (k f) -> p f k", f=D)
                    nc.vector.reduce_sum(
                        out=agg[:], in_=gT, axis=mybir.AxisListType.X
                    )

                    if layer == 3:
                        # pre = agg + r, rounded to bf16 on the way out
                        ob = wpool.tile([P, D], bf, tag="ob")
                        nc.vector.tensor_add(
                            out=ob[:],
                            in0=agg[:],
                            in1=r_res[:, jb * D : (jb + 1) * D],
                        )
                        nc.sync.dma_start(
                            out=out_t.ap()[jb * P : (jb + 1) * P, :], in_=ob[:]
                        )
                        continue

                    # pre = agg + r
                    pre = wpool.tile([P, D], f32, tag="pre")
                    nc.vector.tensor_add(
                        out=pre[:],
                        in0=agg[:],
                        in1=r_res[:, jb * D : (jb + 1) * D],
                    )
                    # hT = relu(pre).T  via PE transpose + ACT evacuation
                    preT = ppool.tile([D, P], f32, tag="preT", bufs=2)
                    nc.tensor.transpose(out=preT[:], in_=pre[:], identity=ident[:])
                    hT = wpool.tile([D, P], f32, tag="hT")
                    nc.scalar.activation(
                        out=hT[:], in_=preT[:], func=mybir.ActivationFunctionType.Relu
                    )
                    # y_next = h @ W_rel (node-major direct: lhsT = hT)
                    ynp = ppool.tile([P, D], f32, tag="ynp", bufs=2)
                    nc.tensor.matmul(
                        out=ynp[:], lhsT=hT[:], rhs=W_rel_nxt[:], start=True, stop=True
                    )
                    ysb = wpool.tile([P, D], f32, tag="ysb")
                    nc.scalar.activation(
                        out=ysb[:], in_=ynp[:], func=mybir.ActivationFunctionType.Copy
                    )
                    nc.sync.dma_start(
                        out=y_own[jb * P : (jb + 1) * P, :], in_=ysb[:]
                    )
                    # r_next = h @ W_root + b (bias via DVE during PSUM evac)
                    rnp = ppool.tile([P, D], f32, tag="rnp", bufs=2)
                    nc.tensor.matmul(
                        out=rnp[:], lhsT=hT[:], rhs=W_root_nxt[:], start=True, stop=True
                    )
                    nc.vector.tensor_add(
                        out=r_res[:, jb * D : (jb + 1) * D],
                        in0=rnp[:],
                        in1=b_nxt[:],
                    )

                if layer < 3:
                    nc.gpsimd.collective_compute(
                        "AllGather",
                        mybir.AluOpType.bypass,
                        replica_groups=[list(range(NCORES))],
                        ins=[y_own[:].opt()],
                        outs=[table2[:].opt()],
                    )

    nc.compile()
    return nc


# ---------------------------------------------------------------- entry


def _prep_and_build(inputs):
    prep = _preprocess(inputs["x"], inputs["edge_index"], inputs["edge_weight"])
    nc = _build(prep)
    W = {
        k: np.ascontiguousarray(np.asarray(inputs[k], dtype=np.float32))
        for k in ("W_rel1", "W_root1", "W_rel2", "W_root2", "W_rel3", "W_root3")
    }
    bb = {
        f"b{i}": np.ascontiguousarray(
            np.broadcast_to(
                np.asarray(inputs[f"b_rel{i}"], dtype=np.float32)[None, :], (P, D)
            )
        )
        for i in (1, 2, 3)
    }
    in_maps = []
    for c in range(NCORES):
        m = {
            "xcore": prep["x_arr"][c],
            "idx_tok": np.ascontiguousarray(prep["idx_tok"][c]),
            "ell_w": np.ascontiguousarray(prep["ell_w"][c]),
        }
        m.update(W)
        m.update(bb)
        in_maps.append(m)
    return prep, nc, in_maps


def _reassemble(prep, core_outs):
    N = prep["N"]
    B = prep["B"]
    perm = prep["perm"]
    out = np.zeros((N, D), dtype=np.float32)
    for c in range(NCORES):
        pr = perm[c * B * P : (c + 1) * B * P]
        real = pr >= 0
        out[pr[real]] = core_outs[c][real].astype(np.float32)
    return out


def kernel(**inputs) -> np.ndarray:
    from concourse.bass_utils import run_bass_kernel_spmd

    prep, nc, in_maps = _prep_and_build(inputs)
    res = run_bass_kernel_spmd(
        nc,
        in_maps,
        core_ids=list(range(NCORES)),
        trace=bool(int(os.environ.get("GCN_TRACE", "0"))),
    )
    kernel.last_results = res
    kernel.last_nc = nc
    kernel.last_in_maps = in_maps
    return _reassemble(prep, [res.results[c]["h3"] for c in range(NCORES)])


if __name__ == "__main__":
    import reference

    inputs = {k: np.asarray(v) for k, v in reference.setup_inputs().items()}
    expected = np.asarray(reference.reference(**inputs))
    actual = kernel(**inputs)
    err = np.abs(actual - expected).max() / (np.abs(expected).max() + 1e-9)
    rel = np.linalg.norm(actual - expected) / (np.linalg.norm(expected) + 1e-30)
    print("max-abs-rel:", err, " fro-rel:", rel)


# revision 4
# speedup vs baseline: 1.2367x; 1.2367x over previous
"""GCNEncoder (3x GraphConv, D=64) on 8 Trainium2 NeuronCores.

Strategy (v3 — transfer-minimized; the axon tunnel at ~70MB/s h2d and
~30MB/s d2h dominates wall-clock, so every input is shipped once, small,
and in the narrowest dtype that keeps rel-err under tolerance):
  - Host: dedup edges, relabel nodes by in-degree (descending), partition the
    relabeled dst nodes into 128-row blocks dealt round-robin across 8 cores,
    and build a block-ELL structure (per dst-block: K_j neighbor slots per
    node, uniform across cores so a single SPMD program works).
  - Quad-packed gather: the f32 y table is viewed as [Npad/4, 4*D] (1024B
    rows), so a gather token addresses a QUAD of nodes. Only Npad/4 = 25088
    token values exist, which fits one signed-int16 window — no 32768-row
    window splitting, so ELL padding is just max-in-degree per 1024-node
    block (degree-sorted, so near-mean). A shipped int8 selector (src % 4)
    is expanded ONCE on device into a [P, 4*K] masked weight table
    (is_equal against an iota pattern), after which each layer's
    multiply+reduce treats the gathered [P, 4K, D] exactly like plain ELL.
  - Per-core inputs: x shard (node-major bf16), tokens [16, T/16] i16
    (replicated to 128 partitions on device), ELL weights bf16 + selector
    int8, weight matrices + pre-broadcast biases f32. Output bf16.
  - Linearity: agg @ W_rel == segment_sum(w * (h @ W_rel)[src]), so each layer
    keeps a node-major table y = h @ W_rel in HBM, and the aggregation output
    plus the root term r = h @ W_root + b is already the layer output.
  - Device prologue (layer-1 dense part): per block, transpose the x block on
    the PE, then y1 = x@W_rel1 and r1 = x@W_root1 + b1 via two matmuls; an
    AllGather builds the full f32 y table. Two more AllGathers rebuild it
    after layers 1 and 2.
"""

import os

import numpy as np

P = 128
D = 64
NCORES = 8


# ---------------------------------------------------------------- host prep


def _preprocess(x, edge_index, edge_weight):
    import ml_dtypes

    bf16 = ml_dtypes.bfloat16
    N = x.shape[0]
    src = np.asarray(edge_index[0], dtype=np.int64)
    dst = np.asarray(edge_index[1], dtype=np.int64)
    w = np.asarray(edge_weight, dtype=np.float64)

    # dedup parallel edges (sum weights)
    key = dst * N + src
    ukey, inv = np.unique(key, return_inverse=True)
    uw = np.bincount(inv, weights=w).astype(np.float32)
    udst = (ukey // N).astype(np.int64)
    usrc = (ukey % N).astype(np.int64)

    deg = np.bincount(udst, minlength=N)

    # per-core block count
    B = -(-N // (NCORES * P))  # ceil
    Npad = NCORES * B * P

    # order nodes by degree desc; sorted position t -> orig node order[t]
    order = np.argsort(-deg, kind="stable")
    order_pad = np.concatenate([order, np.full(Npad - N, -1, dtype=np.int64)])

    # sorted block g = j*NCORES + c  ->  core c, slot j
    # new id layout: new = c*B*P + j*P + p  where sorted pos t = g*P + p
    t = np.arange(Npad)
    g = t // P
    p = t % P
    c = g % NCORES
    j = g // NCORES
    newpos_of_sorted = c * (B * P) + j * P + p
    # perm: new id -> orig node (-1 for dummy)
    perm = np.empty(Npad, dtype=np.int64)
    perm[newpos_of_sorted] = order_pad
    # inv_new: orig node -> new id
    sorted_pos = np.empty(N, dtype=np.int64)
    sorted_pos[order] = np.arange(N)
    inv_new = newpos_of_sorted[sorted_pos]

    nd = inv_new[udst]  # new dst id per edge
    ns = inv_new[usrc]  # new src id per edge

    ej_all = (nd % (B * P)) // P
    ep_all = nd % P
    ec_all = nd // (B * P)
    # counts per (core, slot j, partition)
    cnt = np.zeros((NCORES, B, P), dtype=np.int64)
    np.add.at(cnt, (ec_all, ej_all, ep_all), 1)
    K_j = np.maximum(cnt.max(axis=(0, 2)), 1)  # [B] slots per block
    off_j = np.concatenate([[0], np.cumsum(K_j)])
    K_total = int(off_j[-1])

    # rank of each edge within its dst group
    eorder = np.argsort(nd, kind="stable")
    nd_s = nd[eorder]
    ns_s = ns[eorder]
    w_s = uw[eorder]
    first = np.concatenate([[True], nd_s[1:] != nd_s[:-1]])
    gid = np.cumsum(first) - 1
    gstart = np.nonzero(first)[0]
    k_within = np.arange(len(nd_s)) - gstart[gid]

    ec = nd_s // (B * P)
    rem = nd_s % (B * P)
    ej = rem // P
    ep = rem % P
    col = off_j[ej] + k_within

    ell_idx = np.zeros((NCORES, P, K_total), dtype=np.int16)  # quad ids
    ell_sel = np.zeros((NCORES, P, K_total), dtype=np.int8)  # src % 4
    ell_w = np.zeros((NCORES, P, K_total), dtype=np.float32)
    ell_idx[ec, ep, col] = (ns_s // 4).astype(np.int16)
    ell_sel[ec, ep, col] = (ns_s % 4).astype(np.int8)
    ell_w[ec, ep, col] = w_s

    # token-format (wrapped int16) index arrays for dma_gather:
    # per block j: tokens t = k*128 + p over its column range; wrapped
    # [16, ntok/16]. Shipped non-replicated; the device copies the 16-row
    # strip to all eight 16-partition gpsimd groups.
    tok_cum = np.concatenate([[0], np.cumsum(K_j * P)])
    TOK_TOTAL = int(tok_cum[-1])
    idx_tok = np.zeros((NCORES, 16, TOK_TOTAL // 16), dtype=np.int16)
    for jb in range(B):
        K = int(K_j[jb])
        c0 = int(off_j[jb])
        t0 = int(tok_cum[jb])
        ntok = K * P
        blk = ell_idx[:, :, c0 : c0 + K]  # [NCORES, P, K]
        lin = blk.transpose(0, 2, 1).reshape(NCORES, ntok)  # t = k*128+p
        idx_tok[:, :, t0 // 16 : (t0 + ntok) // 16] = lin.reshape(
            NCORES, ntok // 16, 16
        ).transpose(0, 2, 1)

    # per-core x shard, node-major, bf16
    real = perm >= 0
    x_new = np.zeros((Npad, D), dtype=np.float32)
    x_new[real] = np.asarray(x, dtype=np.float32)[perm[real]]
    x_arr = np.ascontiguousarray(x_new.reshape(NCORES, B * P, D).astype(bf16))

    return dict(
        N=N,
        B=B,
        Npad=Npad,
        perm=perm,
        K_j=K_j,
        off_j=off_j,
        tok_cum=tok_cum,
        TOK_TOTAL=TOK_TOTAL,
        K_total=K_total,
        idx_tok=idx_tok,
        ell_sel=ell_sel,
        ell_w=ell_w.astype(bf16),
        x_arr=x_arr,
    )


# ---------------------------------------------------------------- bass build


def _build(prep):
    import concourse.bacc as bacc
    import concourse.mybir as mybir
    import concourse.tile as tile
    from concourse.masks import make_identity

    f32 = mybir.dt.float32
    bf = mybir.dt.bfloat16
    i16 = mybir.dt.int16
    i8 = mybir.dt.int8
    B = prep["B"]
    Npad = prep["Npad"]
    K_j = prep["K_j"]
    off_j = prep["off_j"]
    tok_cum = prep["tok_cum"]
    TOK_TOTAL = prep["TOK_TOTAL"]
    K_total = prep["K_total"]
    QD = 4 * D  # quad row width

    nc = bacc.Bacc(
        "TRN2",
        target_bir_lowering=False,
        debug=False,
        num_devices=NCORES,
    )

    # IO
    x_in = nc.dram_tensor("xcore", [B * P, D], bf, kind="ExternalInput")
    idx_in = nc.dram_tensor("idx_tok", [16, TOK_TOTAL // 16], i16, kind="ExternalInput")
    w_in = nc.dram_tensor("ell_w", [P, K_total], bf, kind="ExternalInput")
    sel_in = nc.dram_tensor("ell_sel", [P, K_total], i8, kind="ExternalInput")
    wmat_in = {
        nm: nc.dram_tensor(nm, [D, D], f32, kind="ExternalInput")
        for nm in ("W_rel1", "W_root1", "W_rel2", "W_root2", "W_rel3", "W_root3")
    }
    bb_in = {
        nm: nc.dram_tensor(nm, [P, D], f32, kind="ExternalInput")
        for nm in ("b1", "b2", "b3")
    }
    out_t = nc.dram_tensor("h3", [B * P, D], bf, kind="ExternalOutput")

    with tile.TileContext(nc) as tc:
        with (
            tc.tile_pool(name="const", bufs=1) as cpool,
            tc.tile_pool(name="dram", bufs=1, space="DRAM") as dpool,
            tc.tile_pool(name="gather", bufs=3) as gpool,
            tc.tile_pool(name="work", bufs=4) as wpool,
            tc.tile_pool(name="psum", bufs=1, space="PSUM") as ppool,
        ):
            # residents
            idx_res = cpool.tile([P, TOK_TOTAL // 16], i16, tag="idx")
            w_bf = cpool.tile([P, K_total], bf, tag="wbf")
            w_res = cpool.tile([P, K_total], f32, tag="w")
            sel_res = cpool.tile([P, K_total], i8, tag="sel")
            sel4 = cpool.tile([P, 4 * K_total], f32, tag="sel4")
            w4 = cpool.tile([P, 4 * K_total], f32, tag="w4")
            q4 = cpool.tile([P, 4], f32, tag="q4")
            r_res = cpool.tile([P, B * D], f32, tag="r")
            ident = cpool.tile([P, P], f32, tag="ident")
            Wt = {k: cpool.tile([D, D], f32, tag=k, name=k) for k in wmat_in}
            bt = {k: cpool.tile([P, D], f32, tag=k, name=k) for k in bb_in}

            # replicate the 16-row token strip to all 8 gpsimd groups
            for grp in range(8):
                nc.sync.dma_start(
                    out=idx_res[16 * grp : 16 * (grp + 1), :], in_=idx_in.ap()
                )
            nc.sync.dma_start(out=w_bf[:], in_=w_in.ap())
            nc.scalar.activation(
                out=w_res[:], in_=w_bf[:], func=mybir.ActivationFunctionType.Copy
            )
            nc.sync.dma_start(out=sel_res[:], in_=sel_in.ap())
            for k in Wt:
                nc.sync.dma_start(out=Wt[k][:], in_=wmat_in[k].ap())
            for k in bt:
                nc.sync.dma_start(out=bt[k][:], in_=bb_in[k].ap())
            make_identity(nc, ident[:])

            # expand (w, sel) -> w4[p, 4k+s] = w[p,k] * (sel[p,k]==s)
            for s in range(4):
                nc.vector.memset(q4[:, s : s + 1], float(s))
            sel4v = sel4[:].rearrange("p (k s) -> p k s", s=4)
            nc.vector.tensor_copy(
                out=sel4v,
                in_=sel_res[:].unsqueeze(-1).to_broadcast([P, K_total, 4]),
            )
            nc.vector.tensor_tensor(
                out=sel4v,
                in0=sel4v,
                in1=q4[:].unsqueeze(1).to_broadcast([P, K_total, 4]),
                op=mybir.AluOpType.is_equal,
            )
            nc.vector.tensor_tensor(
                out=w4[:].rearrange("p (k s) -> p k s", s=4),
                in0=sel4v,
                in1=w_res[:].unsqueeze(-1).to_broadcast([P, K_total, 4]),
                op=mybir.AluOpType.mult,
            )

            # DRAM: y table (viewed as quads) + own-shard staging
            table2 = dpool.tile([Npad // 4, QD], f32, tag="table")
            y_own = dpool.tile([B * P, D], f32, tag="yown")

            # ---------------- prologue: y1 = x@W_rel1, r1 = x@W_root1 + b1
            for jb in range(B):
                xb = wpool.tile([P, D], bf, tag="xb")
                nc.sync.dma_start(
                    out=xb[:], in_=x_in.ap()[jb * P : (jb + 1) * P, :]
                )
                xf = wpool.tile([P, D], f32, tag="xf")
                nc.scalar.activation(
                    out=xf[:], in_=xb[:], func=mybir.ActivationFunctionType.Copy
                )
                xTp = ppool.tile([D, P], f32, tag="preT", bufs=2)
                nc.tensor.transpose(out=xTp[:], in_=xf[:], identity=ident[:])
                xT = wpool.tile([D, P], f32, tag="hT")
                nc.scalar.activation(
                    out=xT[:], in_=xTp[:], func=mybir.ActivationFunctionType.Copy
                )
                yp = ppool.tile([P, D], f32, tag="ynp", bufs=2)
                nc.tensor.matmul(
                    out=yp[:], lhsT=xT[:], rhs=Wt["W_rel1"][:], start=True, stop=True
                )
                ys = wpool.tile([P, D], f32, tag="ysb")
                nc.scalar.activation(
                    out=ys[:], in_=yp[:], func=mybir.ActivationFunctionType.Copy
                )
                nc.sync.dma_start(out=y_own[jb * P : (jb + 1) * P, :], in_=ys[:])
                rp = ppool.tile([P, D], f32, tag="rnp", bufs=2)
                nc.tensor.matmul(
                    out=rp[:], lhsT=xT[:], rhs=Wt["W_root1"][:], start=True, stop=True
                )
                nc.vector.tensor_add(
                    out=r_res[:, jb * D : (jb + 1) * D], in0=rp[:], in1=bt["b1"][:]
                )

            nc.gpsimd.collective_compute(
                "AllGather",
                mybir.AluOpType.bypass,
                replica_groups=[list(range(NCORES))],
                ins=[y_own[:].opt()],
                outs=[table2[:].opt()],
            )

            # ---------------- 3 gather/aggregate layers
            for layer in (1, 2, 3):
                W_rel_nxt = Wt[f"W_rel{layer + 1}"] if layer < 3 else None
                W_root_nxt = Wt[f"W_root{layer + 1}"] if layer < 3 else None
                b_nxt = bt[f"b{layer + 1}"] if layer < 3 else None

                for jb in range(B):
                    K = int(K_j[jb])
                    off = int(off_j[jb])
                    ntok = K * P
                    t0 = int(tok_cum[jb])
                    g = gpool.tile([P, K * QD], f32, tag="g")
                    nc.gpsimd.dma_gather(
                        out_ap=g[:].rearrange("p (c e) -> p c e", e=QD),
                        in_ap=table2[:],
                        idxs_ap=idx_res[:, t0 // 16 : (t0 + ntok) // 16],
                        num_idxs=ntok,
                        num_idxs_reg=ntok,
                        elem_size=QD,
                        single_packet=False,
                    )
                    # g *= w4 (broadcast along feature dim); slots are 4K wide
                    g3 = g[:].rearrange("p (k f) -> p k f", f=D)
                    wb = (
                        w4[:, 4 * off : 4 * (off + K)]
                        .unsqueeze(-1)
                        .to_broadcast([P, 4 * K, D])
                    )
                    nc.vector.tensor_tensor(
                        out=g3, in0=g3, in1=wb, op=mybir.AluOpType.mult
                    )
                    # agg[p, f] = sum_k g[p, k, f]
                    agg = wpool.tile([P, D], f32, tag="agg")
                    gT = g[:].rearrange("p (k f) -> p f k", f=D)
                    nc.vector.reduce_sum(
                        out=agg[:], in_=gT, axis=mybir.AxisListType.X
                    )

                    if layer == 3:
                        # pre = agg + r, rounded to bf16 on the way out
                        ob = wpool.tile([P, D], bf, tag="ob")
                        nc.vector.tensor_add(
                            out=ob[:],
                            in0=agg[:],
                            in1=r_res[:, jb * D : (jb + 1) * D],
                        )
                        nc.sync.dma_start(
                            out=out_t.ap()[jb * P : (jb + 1) * P, :], in_=ob[:]
                        )
                        continue

                    # pre = agg + r
                    pre = wpool.tile([P, D], f32, tag="pre")
                    nc.vector.tensor_add(
                        out=pre[:],
                        in0=agg[:],
                        in1=r_res[:, jb * D : (jb + 1) * D],
                    )
                    # hT = relu(pre).T  via PE transpose + ACT evacuation
                    preT = ppool.tile([D, P], f32, tag="preT", bufs=2)
                    nc.tensor.transpose(out=preT[:], in_=pre[:], identity=ident[:])
                    hT = wpool.tile([D, P], f32, tag="hT")
                    nc.scalar.activation(
                        out=hT[:], in_=preT[:], func=mybir.ActivationFunctionType.Relu
                    )
                    # y_next = h @ W_rel (node-major direct: lhsT = hT)
                    ynp = ppool.tile([P, D], f32, tag="ynp", bufs=2)
                    nc.tensor.matmul(
                        out=ynp[:], lhsT=hT[:], rhs=W_rel_nxt[:], start=True, stop=True
                    )
                    ysb = wpool.tile([P, D], f32, tag="ysb")
                    nc.scalar.activation(
                        out=ysb[:], in_=ynp[:], func=mybir.ActivationFunctionType.Copy
                    )
                    nc.sync.dma_start(
                        out=y_own[jb * P : (jb + 1) * P, :], in_=ysb[:]
                    )
                    # r_next = h @ W_root + b (bias via DVE during PSUM evac)
                    rnp = ppool.tile([P, D], f32, tag="rnp", bufs=2)
                    nc.tensor.matmul(
                        out=rnp[:], lhsT=hT[:], rhs=W_root_nxt[:], start=True, stop=True
                    )
                    nc.vector.tensor_add(
                        out=r_res[:, jb * D : (jb + 1) * D],
                        in0=rnp[:],
                        in1=b_nxt[:],
                    )

                if layer < 3:
                    nc.gpsimd.collective_compute(
                        "AllGather",
                        mybir.AluOpType.bypass,
                        replica_groups=[list(range(NCORES))],
                        ins=[y_own[:].opt()],
                        outs=[table2[:].opt()],
                    )

    nc.compile()
    return nc


# ---------------------------------------------------------------- entry


def _prep_and_build(inputs):
    prep = _preprocess(inputs["x"], inputs["edge_index"], inputs["edge_weight"])
    nc = _build(prep)
    W = {
        k: np.ascontiguousarray(np.asarray(inputs[k], dtype=np.float32))
        for k in ("W_rel1", "W_root1", "W_rel2", "W_root2", "W_rel3", "W_root3")
    }
    bb = {
        f"b{i}": np.ascontiguousarray(
            np.broadcast_to(
                np.asarray(inputs[f"b_rel{i}"], dtype=np.float32)[None, :], (P, D)
            )
        )
        for i in (1, 2, 3)
    }
    in_maps = []
    for c in range(NCORES):
        m = {
            "xcore": prep["x_arr"][c],
            "idx_tok": np.ascontiguousarray(prep["idx_tok"][c]),
            "ell_w": np.ascontiguousarray(prep["ell_w"][c]),
            "ell_sel": np.ascontiguousarray(prep["ell_sel"][c]),
        }
        m.update(W)
        m.update(bb)
        in_maps.append(m)
    return prep, nc, in_maps


def _reassemble(prep, core_outs):
    N = prep["N"]
    B = prep["B"]
    perm = prep["perm"]
    out = np.zeros((N, D), dtype=np.float32)
    for c in range(NCORES):
        pr = perm[c * B * P : (c + 1) * B * P]
        real = pr >= 0
        out[pr[real]] = core_outs[c][real].astype(np.float32)
    return out


def kernel(**inputs) -> np.ndarray:
    from concourse.bass_utils import run_bass_kernel_spmd

    prep, nc, in_maps = _prep_and_build(inputs)
    res = run_bass_kernel_spmd(
        nc,
        in_maps,
        core_ids=list(range(NCORES)),
        trace=bool(int(os.environ.get("GCN_TRACE", "0"))),
    )
    kernel.last_results = res
    kernel.last_nc = nc
    kernel.last_in_maps = in_maps
    return _reassemble(prep, [res.results[c]["h3"] for c in range(NCORES)])


if __name__ == "__main__":
    import reference

    inputs = {k: np.asarray(v) for k, v in reference.setup_inputs().items()}
    expected = np.asarray(reference.reference(**inputs))
    actual = kernel(**inputs)
    err = np.abs(actual - expected).max() / (np.abs(expected).max() + 1e-9)
    rel = np.linalg.norm(actual - expected) / (np.linalg.norm(expected) + 1e-30)
    print("max-abs-rel:", err, " fro-rel:", rel)


# revision 15
# speedup vs baseline: 1.4434x; 1.1672x over previous
"""GCNEncoder (3x GraphConv, D=64) on 8 Trainium2 NeuronCores.

Strategy (v3 — transfer-minimized; the axon tunnel at ~70MB/s h2d and
~30MB/s d2h dominates wall-clock, so every input is shipped once, small,
and in the narrowest dtype that keeps rel-err under tolerance):
  - Host: dedup edges, relabel nodes by in-degree (descending), partition the
    relabeled dst nodes into 128-row blocks dealt round-robin across 8 cores,
    and build a block-ELL structure (per dst-block: K_j neighbor slots per
    node, uniform across cores so a single SPMD program works).
  - Quad-packed gather: the f32 y table is viewed as [Npad/4, 4*D] (1024B
    rows), so a gather token addresses a QUAD of nodes. Only Npad/4 = 25088
    token values exist, which fits one signed-int16 window — no 32768-row
    window splitting, so ELL padding is just max-in-degree per 1024-node
    block (degree-sorted, so near-mean). A shipped int8 selector (src % 4)
    is expanded ONCE on device into a [P, 4*K] masked weight table
    (is_equal against an iota pattern), after which each layer's
    multiply+reduce treats the gathered [P, 4K, D] exactly like plain ELL.
  - Per-core inputs: x shard (node-major bf16), tokens [16, T/16] i16
    (replicated to 128 partitions on device), ELL weights bf16 + selector
    int8, weight matrices + pre-broadcast biases f32. Output bf16.
  - Linearity: agg @ W_rel == segment_sum(w * (h @ W_rel)[src]), so each layer
    keeps a node-major table y = h @ W_rel in HBM, and the aggregation output
    plus the root term r = h @ W_root + b is already the layer output.
  - Device prologue (layer-1 dense part): per block, transpose the x block on
    the PE, then y1 = x@W_rel1 and r1 = x@W_root1 + b1 via two matmuls; an
    AllGather builds the full f32 y table. Two more AllGathers rebuild it
    after layers 1 and 2.
"""

import os

import numpy as np

P = 128
D = 64
NCORES = 8


# ---------------------------------------------------------------- host prep


def _preprocess(x, edge_index, edge_weight):
    import ml_dtypes

    bf16 = ml_dtypes.bfloat16
    N = x.shape[0]
    src = np.asarray(edge_index[0], dtype=np.int64)
    dst = np.asarray(edge_index[1], dtype=np.int64)
    w = np.asarray(edge_weight, dtype=np.float64)

    # dedup parallel edges (sum weights)
    key = dst * N + src
    ukey, inv = np.unique(key, return_inverse=True)
    uw = np.bincount(inv, weights=w).astype(np.float32)
    udst = (ukey // N).astype(np.int64)
    usrc = (ukey % N).astype(np.int64)

    deg = np.bincount(udst, minlength=N)

    # per-core block count
    B = -(-N // (NCORES * P))  # ceil
    Npad = NCORES * B * P

    # order nodes by degree desc; sorted position t -> orig node order[t]
    order = np.argsort(-deg, kind="stable")
    order_pad = np.concatenate([order, np.full(Npad - N, -1, dtype=np.int64)])

    # sorted block g = j*NCORES + c  ->  core c, slot j
    # new id layout: new = c*B*P + j*P + p  where sorted pos t = g*P + p
    t = np.arange(Npad)
    g = t // P
    p = t % P
    c = g % NCORES
    j = g // NCORES
    newpos_of_sorted = c * (B * P) + j * P + p
    # perm: new id -> orig node (-1 for dummy)
    perm = np.empty(Npad, dtype=np.int64)
    perm[newpos_of_sorted] = order_pad
    # inv_new: orig node -> new id
    sorted_pos = np.empty(N, dtype=np.int64)
    sorted_pos[order] = np.arange(N)
    inv_new = newpos_of_sorted[sorted_pos]

    nd = inv_new[udst]  # new dst id per edge
    ns = inv_new[usrc]  # new src id per edge

    ej_all = (nd % (B * P)) // P
    ep_all = nd % P
    ec_all = nd // (B * P)
    # counts per (core, slot j, partition)
    cnt = np.zeros((NCORES, B, P), dtype=np.int64)
    np.add.at(cnt, (ec_all, ej_all, ep_all), 1)
    K_j = np.maximum(cnt.max(axis=(0, 2)), 1)  # [B] slots per block
    off_j = np.concatenate([[0], np.cumsum(K_j)])
    K_total = int(off_j[-1])

    # rank of each edge within its dst group
    eorder = np.argsort(nd, kind="stable")
    nd_s = nd[eorder]
    ns_s = ns[eorder]
    w_s = uw[eorder]
    first = np.concatenate([[True], nd_s[1:] != nd_s[:-1]])
    gid = np.cumsum(first) - 1
    gstart = np.nonzero(first)[0]
    k_within = np.arange(len(nd_s)) - gstart[gid]

    ec = nd_s // (B * P)
    rem = nd_s % (B * P)
    ej = rem // P
    ep = rem % P
    col = off_j[ej] + k_within

    ell_idx = np.zeros((NCORES, P, K_total), dtype=np.int16)  # quad ids
    ell_sel = np.zeros((NCORES, P, K_total), dtype=np.int8)  # src % 4
    ell_w = np.zeros((NCORES, P, K_total), dtype=np.float32)
    ell_idx[ec, ep, col] = (ns_s // 4).astype(np.int16)
    ell_sel[ec, ep, col] = (ns_s % 4).astype(np.int8)
    ell_w[ec, ep, col] = w_s

    # token-format (wrapped int16) index arrays for dma_gather:
    # per block j: tokens t = k*128 + p over its column range; wrapped
    # [16, ntok/16]. Shipped non-replicated; the device copies the 16-row
    # strip to all eight 16-partition gpsimd groups.
    tok_cum = np.concatenate([[0], np.cumsum(K_j * P)])
    TOK_TOTAL = int(tok_cum[-1])
    idx_tok = np.zeros((NCORES, 16, TOK_TOTAL // 16), dtype=np.int16)
    for jb in range(B):
        K = int(K_j[jb])
        c0 = int(off_j[jb])
        t0 = int(tok_cum[jb])
        ntok = K * P
        blk = ell_idx[:, :, c0 : c0 + K]  # [NCORES, P, K]
        lin = blk.transpose(0, 2, 1).reshape(NCORES, ntok)  # t = k*128+p
        idx_tok[:, :, t0 // 16 : (t0 + ntok) // 16] = lin.reshape(
            NCORES, ntok // 16, 16
        ).transpose(0, 2, 1)

    # per-core x shard, node-major, int8 with per-column scale
    real = perm >= 0
    x_new = np.zeros((Npad, D), dtype=np.float32)
    x_new[real] = np.asarray(x, dtype=np.float32)[perm[real]]
    sx = np.abs(x_new).max(axis=0) / 127.0  # [D]
    sx = np.where(sx == 0, 1.0, sx)
    x_q = np.clip(np.round(x_new / sx[None, :]), -127, 127).astype(np.int8)
    x_arr = np.ascontiguousarray(x_q.reshape(NCORES, B * P, D))

    return dict(
        N=N,
        B=B,
        Npad=Npad,
        perm=perm,
        K_j=K_j,
        off_j=off_j,
        tok_cum=tok_cum,
        TOK_TOTAL=TOK_TOTAL,
        K_total=K_total,
        idx_tok=idx_tok,
        ell_sel=ell_sel,
        ell_w=ell_w.astype(bf16),
        x_arr=x_arr,
        sx=np.ascontiguousarray(sx.astype(np.float32).reshape(D, 1)),
    )


# ---------------------------------------------------------------- bass build


def _build(prep):
    import concourse.bacc as bacc
    import concourse.mybir as mybir
    import concourse.tile as tile
    from concourse.masks import make_identity

    f32 = mybir.dt.float32
    bf = mybir.dt.bfloat16
    i16 = mybir.dt.int16
    i8 = mybir.dt.int8
    B = prep["B"]
    Npad = prep["Npad"]
    K_j = prep["K_j"]
    off_j = prep["off_j"]
    tok_cum = prep["tok_cum"]
    TOK_TOTAL = prep["TOK_TOTAL"]
    K_total = prep["K_total"]
    QD = 4 * D  # quad row width

    nc = bacc.Bacc(
        "TRN2",
        target_bir_lowering=False,
        debug=False,
        num_devices=NCORES,
    )

    # IO
    x_in = nc.dram_tensor("xcore", [B * P, D], i8, kind="ExternalInput")
    sx_in = nc.dram_tensor("sxcol", [D, 1], f32, kind="ExternalInput")
    idx_in = nc.dram_tensor("idx_tok", [16, TOK_TOTAL // 16], i16, kind="ExternalInput")
    w_in = nc.dram_tensor("ell_w", [P, K_total], bf, kind="ExternalInput")
    sel_in = nc.dram_tensor("ell_sel", [P, K_total], i8, kind="ExternalInput")
    wmat_in = {
        nm: nc.dram_tensor(nm, [D, D], f32, kind="ExternalInput")
        for nm in ("W_rel1", "W_root1", "W_rel2", "W_root2", "W_rel3", "W_root3")
    }
    bb_in = {
        nm: nc.dram_tensor(nm, [P, D], f32, kind="ExternalInput")
        for nm in ("b1", "b2", "b3")
    }
    # h3 is emitted transposed ([D, B*P]) as int8 with a per-column dynamic
    # scale (colmax/127) computed on device and AllReduced across cores.
    out_t = nc.dram_tensor("h3", [D, B * P], i8, kind="ExternalOutput")
    cmax_out = nc.dram_tensor("colmax", [D, 1], f32, kind="ExternalOutput")

    with tile.TileContext(nc) as tc:
        with (
            tc.tile_pool(name="const", bufs=1) as cpool,
            tc.tile_pool(name="dram", bufs=1, space="DRAM") as dpool,
            tc.tile_pool(name="gather", bufs=2) as gpool,
            tc.tile_pool(name="work", bufs=4) as wpool,
            tc.tile_pool(name="psum", bufs=1, space="PSUM") as ppool,
        ):
            # residents
            idx_res = cpool.tile([P, TOK_TOTAL // 16], i16, tag="idx")
            w_bf = cpool.tile([P, K_total], bf, tag="wbf")
            w_res = cpool.tile([P, K_total], f32, tag="w")
            sel_res = cpool.tile([P, K_total], i8, tag="sel")
            w4 = cpool.tile([P, 4 * K_total], f32, tag="w4")
            q4 = cpool.tile([P, 4], f32, tag="q4")
            r_res = cpool.tile([P, B * D], f32, tag="r")
            pre3 = cpool.tile([D, B * P], f32, tag="pre3")
            cmax = cpool.tile([D, B], f32, tag="cmax")
            sx_res = cpool.tile([D, 1], f32, tag="sx")
            scale_res = cpool.tile([D, 1], f32, tag="scale")
            cmr = cpool.tile([D, 1], f32, tag="cmr")
            ident = cpool.tile([P, P], f32, tag="ident")
            Wt = {k: cpool.tile([D, D], f32, tag=k, name=k) for k in wmat_in}
            bt = {k: cpool.tile([P, D], f32, tag=k, name=k) for k in bb_in}

            # replicate the 16-row token strip to all 8 gpsimd groups
            for grp in range(8):
                nc.sync.dma_start(
                    out=idx_res[16 * grp : 16 * (grp + 1), :], in_=idx_in.ap()
                )
            nc.sync.dma_start(out=w_bf[:], in_=w_in.ap())
            nc.scalar.activation(
                out=w_res[:], in_=w_bf[:], func=mybir.ActivationFunctionType.Copy
            )
            nc.sync.dma_start(out=sel_res[:], in_=sel_in.ap())
            nc.sync.dma_start(out=sx_res[:], in_=sx_in.ap())
            for k in Wt:
                nc.sync.dma_start(out=Wt[k][:], in_=wmat_in[k].ap())
            for k in bt:
                nc.sync.dma_start(out=bt[k][:], in_=bb_in[k].ap())
            make_identity(nc, ident[:])

            # expand (w, sel) -> w4[p, 4k+s] = w[p,k] * (sel[p,k]==s)
            for s in range(4):
                nc.vector.memset(q4[:, s : s + 1], float(s))
            w4v = w4[:].rearrange("p (k s) -> p k s", s=4)
            nc.vector.tensor_copy(
                out=w4v,
                in_=sel_res[:].unsqueeze(-1).to_broadcast([P, K_total, 4]),
            )
            nc.vector.tensor_tensor(
                out=w4v,
                in0=w4v,
                in1=q4[:].unsqueeze(1).to_broadcast([P, K_total, 4]),
                op=mybir.AluOpType.is_equal,
            )
            nc.vector.tensor_tensor(
                out=w4v,
                in0=w4v,
                in1=w_res[:].unsqueeze(-1).to_broadcast([P, K_total, 4]),
                op=mybir.AluOpType.mult,
            )

            # DRAM: y table (viewed as quads) + own-shard staging
            table2 = dpool.tile([Npad // 4, QD], f32, tag="table")
            y_own = dpool.tile([B * P, D], f32, tag="yown")
            cm_own = dpool.tile([D, 1], f32, tag="cmown")
            cm_red = dpool.tile([D, 1], f32, tag="cmred")

            # ---------------- prologue: y1 = x@W_rel1, r1 = x@W_root1 + b1
            # x arrives int8 (per-column scales sx); the raw integers are
            # upconverted node-major, transposed on the PE, and the dequant
            # scale is applied per-partition during the PSUM evacuation.
            for jb in range(B):
                xb = wpool.tile([P, D], i8, tag="xb")
                nc.sync.dma_start(
                    out=xb[:], in_=x_in.ap()[jb * P : (jb + 1) * P, :]
                )
                xf = wpool.tile([P, D], f32, tag="xf")
                nc.scalar.activation(
                    out=xf[:], in_=xb[:], func=mybir.ActivationFunctionType.Copy
                )
                xTp = ppool.tile([D, P], f32, tag="preT", bufs=2)
                nc.tensor.transpose(out=xTp[:], in_=xf[:], identity=ident[:])
                xT = wpool.tile([D, P], f32, tag="hT")
                nc.scalar.activation(
                    out=xT[:],
                    in_=xTp[:],
                    func=mybir.ActivationFunctionType.Copy,
                    scale=sx_res[:],
                )
                yp = ppool.tile([P, D], f32, tag="ynp", bufs=2)
                nc.tensor.matmul(
                    out=yp[:], lhsT=xT[:], rhs=Wt["W_rel1"][:], start=True, stop=True
                )
                ys = wpool.tile([P, D], f32, tag="ysb")
                nc.scalar.activation(
                    out=ys[:], in_=yp[:], func=mybir.ActivationFunctionType.Copy
                )
                nc.sync.dma_start(out=y_own[jb * P : (jb + 1) * P, :], in_=ys[:])
                rp = ppool.tile([P, D], f32, tag="rnp", bufs=2)
                nc.tensor.matmul(
                    out=rp[:], lhsT=xT[:], rhs=Wt["W_root1"][:], start=True, stop=True
                )
                nc.vector.tensor_add(
                    out=r_res[:, jb * D : (jb + 1) * D], in0=rp[:], in1=bt["b1"][:]
                )

            nc.gpsimd.collective_compute(
                "AllGather",
                mybir.AluOpType.bypass,
                replica_groups=[list(range(NCORES))],
                ins=[y_own[:].opt()],
                outs=[table2[:].opt()],
            )

            # ---------------- 3 gather/aggregate layers
            for layer in (1, 2, 3):
                W_rel_nxt = Wt[f"W_rel{layer + 1}"] if layer < 3 else None
                W_root_nxt = Wt[f"W_root{layer + 1}"] if layer < 3 else None
                b_nxt = bt[f"b{layer + 1}"] if layer < 3 else None

                for jb in range(B):
                    K = int(K_j[jb])
                    off = int(off_j[jb])
                    ntok = K * P
                    t0 = int(tok_cum[jb])
                    g = gpool.tile([P, K * QD], f32, tag="g")
                    nc.gpsimd.dma_gather(
                        out_ap=g[:].rearrange("p (c e) -> p c e", e=QD),
                        in_ap=table2[:],
                        idxs_ap=idx_res[:, t0 // 16 : (t0 + ntok) // 16],
                        num_idxs=ntok,
                        num_idxs_reg=ntok,
                        elem_size=QD,
                        single_packet=False,
                    )
                    # g *= w4 (broadcast along feature dim); slots are 4K wide
                    g3 = g[:].rearrange("p (k f) -> p k f", f=D)
                    wb = (
                        w4[:, 4 * off : 4 * (off + K)]
                        .unsqueeze(-1)
                        .to_broadcast([P, 4 * K, D])
                    )
                    nc.vector.tensor_tensor(
                        out=g3, in0=g3, in1=wb, op=mybir.AluOpType.mult
                    )
                    # agg[p, f] = sum_k g[p, k, f]
                    agg = wpool.tile([P, D], f32, tag="agg")
                    gT = g[:].rearrange("p (k f) -> p f k", f=D)
                    nc.vector.reduce_sum(
                        out=agg[:], in_=gT, axis=mybir.AxisListType.X
                    )

                    # pre = agg + r
                    pre = wpool.tile([P, D], f32, tag="pre")
                    nc.vector.tensor_add(
                        out=pre[:],
                        in0=agg[:],
                        in1=r_res[:, jb * D : (jb + 1) * D],
                    )

                    if layer == 3:
                        # stash pre.T and its per-column |max| partial; the
                        # int8 emit happens after the cross-core max reduce
                        preT = ppool.tile([D, P], f32, tag="preT", bufs=2)
                        nc.tensor.transpose(
                            out=preT[:], in_=pre[:], identity=ident[:]
                        )
                        nc.scalar.activation(
                            out=pre3[:, jb * P : (jb + 1) * P],
                            in_=preT[:],
                            func=mybir.ActivationFunctionType.Copy,
                        )
                        nc.vector.reduce_max(
                            out=cmax[:, jb : jb + 1],
                            in_=pre3[:, jb * P : (jb + 1) * P],
                            axis=mybir.AxisListType.X,
                            apply_absolute_value=True,
                        )
                        continue
                    # hT = relu(pre).T  via PE transpose + ACT evacuation
                    preT = ppool.tile([D, P], f32, tag="preT", bufs=2)
                    nc.tensor.transpose(out=preT[:], in_=pre[:], identity=ident[:])
                    hT = wpool.tile([D, P], f32, tag="hT")
                    nc.scalar.activation(
                        out=hT[:], in_=preT[:], func=mybir.ActivationFunctionType.Relu
                    )
                    # y_next = h @ W_rel (node-major direct: lhsT = hT)
                    ynp = ppool.tile([P, D], f32, tag="ynp", bufs=2)
                    nc.tensor.matmul(
                        out=ynp[:], lhsT=hT[:], rhs=W_rel_nxt[:], start=True, stop=True
                    )
                    ysb = wpool.tile([P, D], f32, tag="ysb")
                    nc.scalar.activation(
                        out=ysb[:], in_=ynp[:], func=mybir.ActivationFunctionType.Copy
                    )
                    nc.sync.dma_start(
                        out=y_own[jb * P : (jb + 1) * P, :], in_=ysb[:]
                    )
                    # r_next = h @ W_root + b (bias via DVE during PSUM evac)
                    rnp = ppool.tile([P, D], f32, tag="rnp", bufs=2)
                    nc.tensor.matmul(
                        out=rnp[:], lhsT=hT[:], rhs=W_root_nxt[:], start=True, stop=True
                    )
                    nc.vector.tensor_add(
                        out=r_res[:, jb * D : (jb + 1) * D],
                        in0=rnp[:],
                        in1=b_nxt[:],
                    )

                if layer < 3:
                    nc.gpsimd.collective_compute(
                        "AllGather",
                        mybir.AluOpType.bypass,
                        replica_groups=[list(range(NCORES))],
                        ins=[y_own[:].opt()],
                        outs=[table2[:].opt()],
                    )

            # ---------------- epilogue: global colmax -> int8 emit
            # cm1 = (per-core colmax)/127; AllReduce-max; emit scale = 1/cm1.
            # The host dequant scale is then exactly the shipped colmax value.
            cm1 = wpool.tile([D, 1], f32, tag="cm1")
            nc.vector.reduce_max(
                out=cm1[:], in_=cmax[:], axis=mybir.AxisListType.X
            )
            nc.vector.tensor_scalar_mul(out=cm1[:], in0=cm1[:], scalar1=1.0 / 127.0)
            nc.sync.dma_start(out=cm_own[:], in_=cm1[:])
            nc.gpsimd.collective_compute(
                "AllReduce",
                mybir.AluOpType.max,
                replica_groups=[list(range(NCORES))],
                ins=[cm_own[:].opt()],
                outs=[cm_red[:].opt()],
            )
            nc.sync.dma_start(out=cmr[:], in_=cm_red[:])
            nc.sync.dma_start(out=cmax_out.ap(), in_=cm_red[:])
            nc.vector.reciprocal(out=scale_res[:], in_=cmr[:])
            for jb in range(B):
                obi = wpool.tile([D, P], i8, tag="obi")
                nc.scalar.activation(
                    out=obi[:],
                    in_=pre3[:, jb * P : (jb + 1) * P],
                    func=mybir.ActivationFunctionType.Copy,
                    scale=scale_res[:],
                )
                nc.sync.dma_start(
                    out=out_t.ap()[:, jb * P : (jb + 1) * P], in_=obi[:]
                )

    nc.compile()
    return nc


# ---------------------------------------------------------------- entry


def _prep_and_build(inputs):
    prep = _preprocess(inputs["x"], inputs["edge_index"], inputs["edge_weight"])
    nc = _build(prep)
    W = {
        k: np.ascontiguousarray(np.asarray(inputs[k], dtype=np.float32))
        for k in ("W_rel1", "W_root1", "W_rel2", "W_root2", "W_rel3", "W_root3")
    }
    bb = {
        f"b{i}": np.ascontiguousarray(
            np.broadcast_to(
                np.asarray(inputs[f"b_rel{i}"], dtype=np.float32)[None, :], (P, D)
            )
        )
        for i in (1, 2, 3)
    }
    in_maps = []
    for c in range(NCORES):
        m = {
            "xcore": prep["x_arr"][c],
            "sxcol": prep["sx"],
            "idx_tok": np.ascontiguousarray(prep["idx_tok"][c]),
            "ell_w": np.ascontiguousarray(prep["ell_w"][c]),
            "ell_sel": np.ascontiguousarray(prep["ell_sel"][c]),
        }
        m.update(W)
        m.update(bb)
        in_maps.append(m)
    return prep, nc, in_maps


def _reassemble(prep, core_outs, core_cmax):
    N = prep["N"]
    B = prep["B"]
    perm = prep["perm"]
    out = np.zeros((N, D), dtype=np.float32)
    for c in range(NCORES):
        # h3 arrives transposed [D, B*P] int8; colmax is already the
        # dequant scale (global |max|/127)
        sc = core_cmax[c].reshape(D).astype(np.float32)
        h = core_outs[c].astype(np.float32).T * sc[None, :]
        pr = perm[c * B * P : (c + 1) * B * P]
        real = pr >= 0
        out[pr[real]] = h[real]
    return out


def kernel(**inputs) -> np.ndarray:
    from concourse.bass_utils import run_bass_kernel_spmd

    prep, nc, in_maps = _prep_and_build(inputs)
    res = run_bass_kernel_spmd(
        nc,
        in_maps,
        core_ids=list(range(NCORES)),
        trace=bool(int(os.environ.get("GCN_TRACE", "0"))),
    )
    kernel.last_results = res
    kernel.last_nc = nc
    kernel.last_in_maps = in_maps
    return _reassemble(
        prep,
        [res.results[c]["h3"] for c in range(NCORES)],
        [res.results[c]["colmax"] for c in range(NCORES)],
    )


if __name__ == "__main__":
    import reference

    inputs = {k: np.asarray(v) for k, v in reference.setup_inputs().items()}
    expected = np.asarray(reference.reference(**inputs))
    actual = kernel(**inputs)
    err = np.abs(actual - expected).max() / (np.abs(expected).max() + 1e-9)
    rel = np.linalg.norm(actual - expected) / (np.linalg.norm(expected) + 1e-30)
    print("max-abs-rel:", err, " fro-rel:", rel)
